# revision 15
# baseline (speedup 1.0000x reference)
"""Distributed multi-head attention kernel for 8 Trainium2 NeuronCores.

Problem: x[2,2048,768] @ Wqkv[768,2304] + bqkv -> 12-head attention -> @ Wproj + bproj.

Sharding: data-parallel over batch (2) x sequence-quarter (4) = 8 cores, no
collectives. Each core receives xT ROTATED so its own query quarter is the
first 512 columns; softmax over keys is order-invariant, so K/V/scores use
the rotated j-order uniformly (no per-core program divergence).

ScalarE exp is the pacing engine (12 heads x 512 x 2048 exps ~ 110us floor at
1 elem/cycle/lane); the schedule keeps it fed from ~5us on:
- Scores run ROW-PACKED: per head pair one PSUM slot gets two concurrent
  64-contract matmuls at tile_position (0,0)/(64,0) (head A in array rows
  0:64, head B in 64:128), so score MMs cost half the baseline.
- Each exp instruction covers both heads of a pair for one j-tile
  ([128, 2, 512] PSUM -> PT bf16).
- The attention-value matmul keeps the baseline's ones-column trick (V window
  of 128 cols gives 64 ctx rows + denominator row per head, junk never read),
  accumulated per pair into one 2-bank PSUM tile, lagged one pair behind exp
  (within-pair lag 2 for the last pair to kill the tail).
- Denominators: staging row copies + DMA gather to [12, 512], then per-pair
  reciprocal_approx_fast + GpSimd partition_broadcast + one [128,512] DVE
  mult on the packed pair context (normalize-late; projection is linear).
- Projection: pairs 0-3 contraction woven into pair 5 (into an SBUF f32
  accumulator that reuses the freed xT/wqkv space), pairs 4-5 in the tail.
- DMA priority order + lazy K^T/Q^T c-tile emission keep the first exp at
  ~5us; V blocks are the main pair-0/1 filler; K^T c-tiles finish by pair 2
  so the xT/wqkv pool can close.
"""

import numpy as np
import ml_dtypes

B = 2
L = 2048
D = 768
H = 12
HD = 64
SCALE = HD ** -0.5
N_CORES = 8
LQ = L // 4   # 512 query rows per core
LT = L // 128  # 16 key tiles
DT = D // 128  # 6 contraction tiles
NP = H // 2    # 6 head pairs
VW = 65        # V block width per head (64 ctx + 1 ones)
VPAD = 11 * VW + 128 + 5

_CACHED = {}


def _build_nc():
    import concourse.bass as bass
    import concourse.mybir as mybir
    import concourse.tile as tile
    from concourse import bacc, library_config

    F32 = mybir.dt.float32
    BF16 = mybir.dt.bfloat16
    Alu = mybir.AluOpType
    Act = mybir.ActivationFunctionType

    nc = bacc.Bacc(target_bir_lowering=False)

    xT_h = nc.declare_dram_parameter("xT", [D, L], BF16, isOutput=False)
    wqkv_h = nc.declare_dram_parameter("wqkv", [D, 3 * D], BF16, isOutput=False)
    bqkv_h = nc.declare_dram_parameter("bqkv2", [128, 18], F32, isOutput=False)
    wp_h = nc.declare_dram_parameter("wproj2", [128, DT, D], BF16, isOutput=False)
    bp_h = nc.declare_dram_parameter("bproj", [D], F32, isOutput=False)
    bv_h = nc.declare_dram_parameter("bvvec", [D], F32, isOutput=False)
    y_h = nc.declare_dram_parameter("y", [LQ, D], F32, isOutput=True)

    with tile.TileContext(nc) as tc:
        with tc.tile_pool(name="persist", bufs=1) as pp:
            KT_sb = pp.tile([128, DT, L], BF16)    # K^T per pair c-tile
            QT2_sb = pp.tile([128, DT, LQ], BF16)  # Q^T packed pairs
            V_sb = pp.tile([128, LT, VPAD], BF16)  # [V_h | ones] blocks at h*65
            OT2_sb = pp.tile([128, DT, LQ], BF16)  # ctx^T per pair (packed)
            bias_sb = pp.tile([128, 18], F32)
            bv_sb = pp.tile([128, D], F32)
            bp_sb = pp.tile([128, D], F32)
            dst_sb = pp.tile([128, LQ], F32)       # denom staging rows 63/64
            DallA = pp.tile([1, LQ], F32)          # pair denominators (rotating)
            DallB = pp.tile([1, LQ], F32)
            RsbA = pp.tile([1, LQ], F32)           # 1/denom
            RsbB = pp.tile([1, LQ], F32)
            rbA_sb = pp.tile([128, LQ], F32)       # bcast 1/denom (full rows)
            rbB_sb = pp.tile([128, LQ], F32)
            wp_sb = pp.tile([128, DT, D], BF16)

            # partition_broadcast needs the attn gpsimd ucode library
            nc.gpsimd.load_library(library_config.attn)
            # constants
            for h in range(H):
                nc.vector.memset(V_sb[:, :, h * VW + HD:h * VW + HD + 1], 1.0)
            nc.vector.memset(dst_sb[0:1, 0:8], 0.0)
            # preload the exp table set during input DMA
            nc.scalar.activation(dst_sb[0:1, 4:8], dst_sb[0:1, 0:4], Act.Exp)

            with (
                tc.tile_pool(name="ptp", bufs=2) as ptp,
                tc.tile_pool(name="ps_s", bufs=2, space="PSUM") as ps_s,
                tc.tile_pool(name="ps_av", bufs=2, space="PSUM") as ps_av,
            ):
                wq_r = wqkv_h[:].rearrange("(n p) c -> p n c", p=128)
                xT_r = xT_h[:].rearrange("(n p) l -> p n l", p=128)

                def qT_block(kt):
                    ps = ps_s.tile([128, 2, LQ], F32, tag="sps")
                    for dt in range(DT):
                        nc.tensor.matmul(
                            ps[:, 0, :],
                            wqkv_sb[:, dt, kt * 128:(kt + 1) * 128],
                            xT_sb[:, dt, 0:LQ],
                            start=(dt == 0), stop=(dt == DT - 1),
                        )
                    nc.vector.tensor_scalar_add(
                        QT2_sb[:, kt, :], ps[:, 0, :], bias_sb[:, kt:kt + 1])

                def kT_block(kt, lc):
                    ps = ps_s.tile([128, 2, LQ], F32, tag="sps")
                    for dt in range(DT):
                        nc.tensor.matmul(
                            ps[:, 0, :],
                            wqkv_sb[:, dt, D + kt * 128:D + (kt + 1) * 128],
                            xT_sb[:, dt, lc * 512:(lc + 1) * 512],
                            start=(dt == 0), stop=(dt == DT - 1),
                        )
                    nc.vector.tensor_scalar_add(
                        KT_sb[:, kt, lc * 512:(lc + 1) * 512], ps[:, 0, :],
                        bias_sb[:, 6 + kt:7 + kt])

                def vh_block(lt, hf):
                    ps = ps_s.tile([128, 2, LQ], F32, tag="sps")
                    for dt in range(DT):
                        nc.tensor.matmul(
                            ps[:, 0, 0:384],
                            xT_sb[:, dt, lt * 128:(lt + 1) * 128],
                            wqkv_sb[:, dt, 2 * D + hf * 384:2 * D + (hf + 1) * 384],
                            start=(dt == 0), stop=(dt == DT - 1),
                        )
                    nc.vector.tensor_tensor(
                        V_sb[:, lt, 390 * hf:390 * hf + 390].rearrange(
                            "p (h c) -> p h c", c=VW)[:, :, 0:HD],
                        ps[:, 0, 0:384].rearrange("p (h d) -> p h d", h=6),
                        bv_sb[:, hf * 384:(hf + 1) * 384].rearrange(
                            "p (h d) -> p h d", h=6),
                        Alu.add,
                    )

                def s_slot(p, jt, PT):
                    # S^T[j, i] for both heads of the pair, concurrently:
                    # head A in array rows 0:64, head B in rows 64:128
                    ps = ps_s.tile([128, 2, LQ], F32, tag="sps")
                    nc.tensor.matmul(
                        ps[:, 0, :],
                        KT_sb[0:64, p, jt * 128:(jt + 1) * 128],
                        QT2_sb[0:64, p, :],
                        start=True, stop=True, tile_position=(0, 0),
                    )
                    nc.tensor.matmul(
                        ps[:, 1, :],
                        KT_sb[64:128, p, jt * 128:(jt + 1) * 128],
                        QT2_sb[64:128, p, :],
                        start=True, stop=True, tile_position=(64, 0),
                    )
                    nc.scalar.activation(PT[:, jt, :, :], ps, Act.Exp, scale=SCALE)

                def av_mms(p, jt, PT, av):
                    # ones-column windows: even head ctx rows 0:64 + den row 64;
                    # odd head (window shifted 64 left) ctx rows 64:128 + den row 63
                    nc.tensor.matmul(
                        av[:, 0, :],
                        V_sb[:, jt, 2 * p * VW:2 * p * VW + 128],
                        PT[:, jt, 0, :],
                        start=(jt == 0), stop=(jt == LT - 1),
                        skip_group_check=True,
                    )
                    nc.tensor.matmul(
                        av[:, 1, :],
                        V_sb[:, jt, (2 * p + 1) * VW - 64:(2 * p + 1) * VW + 64],
                        PT[:, jt, 1, :],
                        start=(jt == 0), stop=(jt == LT - 1),
                        skip_group_check=True,
                    )

                def av_finish(p, av):
                    nc.vector.tensor_copy(OT2_sb[0:64, p, :], av[0:64, 0, :])
                    nc.vector.tensor_copy(OT2_sb[64:128, p, :], av[64:128, 1, :])
                    nc.vector.tensor_copy(dst_sb[64:65, :], av[64:65, 0, :])
                    # DVE partition base must be 32-aligned; den row 63 rides
                    # along in a 32:64 copy, the DMA below picks out row 63
                    nc.vector.tensor_copy(dst_sb[32:64, :], av[32:64, 1, :])
                    nc.sync.dma_start(out=DallA[:], in_=dst_sb[64:65, :])
                    nc.sync.dma_start(out=DallB[:], in_=dst_sb[63:64, :])

                def normalize(p):
                    # partition_broadcast must target a full 128-row buffer
                    # (partial-range outputs misbehave on HW ucode)
                    nc.vector.reciprocal_approx_fast(RsbA[:], DallA[:])
                    nc.vector.reciprocal_approx_fast(RsbB[:], DallB[:])
                    nc.gpsimd.partition_broadcast(rbA_sb, RsbA[:])
                    nc.gpsimd.partition_broadcast(rbB_sb, RsbB[:])
                    nc.vector.tensor_tensor(
                        OT2_sb[0:64, p, :], OT2_sb[0:64, p, :],
                        rbA_sb[0:64, :], Alu.mult)
                    nc.vector.tensor_tensor(
                        OT2_sb[64:128, p, :], OT2_sb[64:128, p, :],
                        rbB_sb[64:128, :], Alu.mult)

                def proj_group(ic, eh, p0, p1, first):
                    ps = ps_s.tile([128, 2, LQ], F32, tag="sps")
                    for p in range(p0, p1):
                        nc.tensor.matmul(
                            ps[:, 0, 0:384],
                            OT2_sb[:, p, ic * 128:(ic + 1) * 128],
                            wp_sb[:, p, eh * 384:(eh + 1) * 384],
                            start=(p == p0), stop=(p == p1 - 1),
                        )
                    dst = yacc_sb[:, ic, eh * 384:(eh + 1) * 384]
                    if first:
                        nc.vector.tensor_tensor(
                            dst, ps[:, 0, 0:384],
                            bp_sb[:, eh * 384:(eh + 1) * 384], Alu.add)
                    else:
                        nc.vector.tensor_tensor(dst, dst, ps[:, 0, 0:384], Alu.add)

                # ---- phase A: load pool + qkv GEMMs + pairs 0-2 ----
                with tc.tile_pool(name="loadp", bufs=1) as lp:
                    xT_sb = lp.tile([128, DT, L], BF16)
                    wqkv_sb = lp.tile([128, DT, 3 * D], BF16)

                    # DMA priority order (sync queue is FIFO)
                    nc.sync.dma_start(out=bias_sb, in_=bqkv_h[:])
                    nc.sync.dma_start(
                        out=wqkv_sb[:, :, 0:128], in_=wq_r[:, :, 0:128])
                    nc.sync.dma_start(
                        out=xT_sb[:, :, 0:512], in_=xT_r[:, :, 0:512])
                    nc.sync.dma_start(
                        out=wqkv_sb[:, :, D:D + 128], in_=wq_r[:, :, D:D + 128])
                    nc.sync.dma_start(
                        out=xT_sb[:, :, 512:1024], in_=xT_r[:, :, 512:1024])
                    nc.sync.dma_start(
                        out=wqkv_sb[:, :, 128:D], in_=wq_r[:, :, 128:D])
                    nc.sync.dma_start(
                        out=wqkv_sb[:, :, D + 128:2 * D],
                        in_=wq_r[:, :, D + 128:2 * D])
                    nc.sync.dma_start(
                        out=wqkv_sb[:, :, 2 * D:3 * D], in_=wq_r[:, :, 2 * D:3 * D])
                    nc.sync.dma_start(
                        out=xT_sb[:, :, 1024:1536], in_=xT_r[:, :, 1024:1536])
                    nc.sync.dma_start(
                        out=xT_sb[:, :, 1536:2048], in_=xT_r[:, :, 1536:2048])
                    nc.sync.dma_start(out=wp_sb, in_=wp_h[:])
                    bv_src = bv_h[:]
                    nc.gpsimd.dma_start(
                        out=bv_sb,
                        in_=bass.AP(tensor=bv_src.tensor, offset=bv_src.offset,
                                    ap=[[0, 128]] + list(bv_src.ap)),
                    )
                    bp_src = bp_h[:]
                    nc.gpsimd.dma_start(
                        out=bp_sb,
                        in_=bass.AP(tensor=bp_src.tensor, offset=bp_src.offset,
                                    ap=[[0, 128]] + list(bp_src.ap)),
                    )

                    qT_block(0)
                    kT_block(0, 0)

                    # deadline-ordered filler blocks (~6 MMs each), drained at
                    # ~1.3 blocks/slot so exp never waits long on PE fillers
                    blocks = []
                    for kt in range(1, DT):
                        blocks.append((16 * kt - 8, lambda kt=kt: qT_block(kt)))
                    for kt in range(DT):
                        for lc in range(4):
                            if (kt, lc) == (0, 0):
                                continue
                            blocks.append(
                                (16 * kt + 4 * lc,
                                 lambda kt=kt, lc=lc: kT_block(kt, lc)))
                    for lt in range(LT):
                        for hf in range(2):
                            blocks.append(
                                (15 + lt, lambda lt=lt, hf=hf: vh_block(lt, hf)))
                    blocks.sort(key=lambda b: b[0])
                    bq = list(blocks)
                    state = {"emitted": 0}

                    def drain(g, cap):
                        budget = int(1.3 * (g + 2))
                        while bq and (bq[0][0] <= g + 1
                                      or (state["emitted"] < budget
                                          and state["emitted"] < cap)):
                            _, th = bq.pop(0)
                            th()
                            state["emitted"] += 1

                    PTs = {}
                    avs = {}
                    for p in range(3):
                        PTs[p] = ptp.tile([128, LT, 2, LQ], BF16, tag="PT", name=f"PT{p}")
                        if p >= 1:
                            avs[p - 1] = ps_av.tile([128, 2, LQ], F32, tag="av", name=f"av{p-1}")
                        for jt in range(LT):
                            s_slot(p, jt, PTs[p])
                            if p >= 1:
                                av_mms(p - 1, jt, PTs[p - 1], avs[p - 1])
                            drain(16 * p + jt, 10 ** 9)
                        if p >= 1:
                            av_finish(p - 1, avs[p - 1])
                            normalize(p - 1)
                    while bq:
                        _, th = bq.pop(0)
                        th()

                # ---- phase B: pairs 3-5 + projection (xT/wqkv space freed) ----
                with tc.tile_pool(name="yaccp", bufs=1) as yp:
                    yacc_sb = yp.tile([128, 4, D], F32)

                    for p in range(3, NP):
                        PTs[p] = ptp.tile([128, LT, 2, LQ], BF16, tag="PT", name=f"PT{p}")
                        avs[p - 1] = ps_av.tile([128, 2, LQ], F32, tag="av", name=f"av{p-1}")
                        if p == NP - 1:
                            avs[p] = ps_av.tile([128, 2, LQ], F32, tag="av", name=f"av{p}")
                        pg = 0
                        for jt in range(LT):
                            s_slot(p, jt, PTs[p])
                            av_mms(p - 1, jt, PTs[p - 1], avs[p - 1])
                            if p == NP - 1 and jt >= 1:
                                av_mms(p, jt - 1, PTs[p], avs[p])
                            if p == NP - 1 and jt % 2 == 0:
                                # pairs 0-3 projection woven into pair 5
                                proj_group(pg // 2, pg % 2, 0, 4, first=True)
                                pg += 1
                        av_finish(p - 1, avs[p - 1])
                        normalize(p - 1)

                    # tail: finish av(5), normalize, project pairs 4-5, DMA out
                    p = NP - 1
                    av_mms(p, LT - 1, PTs[p], avs[p])
                    av_finish(p, avs[p])
                    normalize(p)
                    y_r = y_h[:].rearrange("(n p) e -> p n e", p=128)
                    for ic in range(4):
                        for eh in range(2):
                            proj_group(ic, eh, 4, 6, first=False)
                        nc.sync.dma_start(out=y_r[:, ic, :], in_=yacc_sb[:, ic, :])

    nc.finalize()
    return nc


def _get_nc():
    if "nc" not in _CACHED:
        _CACHED["nc"] = _build_nc()
    return _CACHED["nc"]


def _make_in_maps(x, Wqkv, bqkv, Wproj, bproj):
    bf16 = ml_dtypes.bfloat16
    x = np.asarray(x, dtype=np.float32)
    wqkv16 = np.ascontiguousarray(np.asarray(Wqkv, dtype=np.float32).astype(bf16))
    bqkv32 = np.asarray(bqkv, dtype=np.float32)
    bqkv2 = np.ascontiguousarray(bqkv32.reshape(18, 128).T)
    bvvec = np.ascontiguousarray(bqkv32[2 * D:3 * D])
    wp2 = np.ascontiguousarray(
        np.asarray(Wproj, dtype=np.float32).astype(bf16)
        .reshape(D // 128, 128, D).transpose(1, 0, 2))
    bp32 = np.ascontiguousarray(np.asarray(bproj, dtype=np.float32))

    xT = [np.ascontiguousarray(x[b].T.astype(bf16)) for b in range(B)]
    in_maps = []
    for c in range(N_CORES):
        b, q = c // 4, c % 4
        # rotate so this core's query quarter is first; softmax over keys is
        # order-invariant, so the rotated j-order is used consistently
        xrot = np.ascontiguousarray(
            np.concatenate([xT[b][:, q * LQ:], xT[b][:, :q * LQ]], axis=1))
        in_maps.append({
            "xT": xrot,
            "wqkv": wqkv16,
            "bqkv2": bqkv2,
            "bvvec": bvvec,
            "wproj2": wp2,
            "bproj": bp32,
        })
    return in_maps


def run(inputs, trace=False):
    """Run the SPMD kernel. Returns (full output [2,2048,768] f32, results)."""
    from concourse.bass_utils import run_bass_kernel_spmd

    nc = _get_nc()
    in_maps = _make_in_maps(**inputs)
    res = run_bass_kernel_spmd(nc, in_maps, list(range(N_CORES)), trace=trace)
    out = np.empty((B, L, D), dtype=np.float32)
    for c in range(N_CORES):
        b, q = c // 4, c % 4
        out[b, q * LQ:(q + 1) * LQ, :] = res.results[c]["y"]
    return out, res


def kernel(**inputs) -> np.ndarray:
    return run(inputs)[0]


# revision 19
# speedup vs baseline: 1.0081x; 1.0081x over previous
"""Distributed multi-head attention kernel for 8 Trainium2 NeuronCores.

Problem: x[2,2048,768] @ Wqkv[768,2304] + bqkv -> 12-head attention -> @ Wproj + bproj.

Sharding: data-parallel over batch (2) x sequence-quarter (4) = 8 cores, no
collectives. Each core receives xT ROTATED so its own query quarter is the
first 512 columns; softmax over keys is order-invariant, so K/V/scores use
the rotated j-order uniformly (no per-core program divergence).

ScalarE exp is the pacing engine (12 heads x 512 x 2048 exps ~ 110us floor at
1 elem/cycle/lane); the schedule keeps it fed from ~5us on:
- Scores run ROW-PACKED: per head pair one PSUM slot gets two concurrent
  64-contract matmuls at tile_position (0,0)/(64,0) (head A in array rows
  0:64, head B in 64:128), so score MMs cost half the baseline.
- Each exp instruction covers both heads of a pair for one j-tile
  ([128, 2, 512] PSUM -> PT bf16).
- The attention-value matmul keeps the baseline's ones-column trick (V window
  of 128 cols gives 64 ctx rows + denominator row per head, junk never read),
  accumulated per pair into one 2-bank PSUM tile, lagged one pair behind exp
  (within-pair lag 2 for the last pair to kill the tail).
- Denominators: staging row copies + DMA gather to [12, 512], then per-pair
  reciprocal_approx_fast + GpSimd partition_broadcast + one [128,512] DVE
  mult on the packed pair context (normalize-late; projection is linear).
- Projection: pairs 0-3 contraction woven into pair 5 (into an SBUF f32
  accumulator that reuses the freed xT/wqkv space), pairs 4-5 in the tail.
- DMA priority order + lazy K^T/Q^T c-tile emission keep the first exp at
  ~5us; V blocks are the main pair-0/1 filler; K^T c-tiles finish by pair 2
  so the xT/wqkv pool can close.
"""

import numpy as np
import ml_dtypes

B = 2
L = 2048
D = 768
H = 12
HD = 64
SCALE = HD ** -0.5
N_CORES = 8
LQ = L // 4   # 512 query rows per core
LT = L // 128  # 16 key tiles
DT = D // 128  # 6 contraction tiles
NP = H // 2    # 6 head pairs
VW = 65        # V block width per head (64 ctx + 1 ones)
VPAD = 11 * VW + 128 + 5

_CACHED = {}


def _build_nc():
    import concourse.bass as bass
    import concourse.mybir as mybir
    import concourse.tile as tile
    from concourse import bacc, library_config

    F32 = mybir.dt.float32
    BF16 = mybir.dt.bfloat16
    Alu = mybir.AluOpType
    Act = mybir.ActivationFunctionType

    nc = bacc.Bacc(target_bir_lowering=False)

    xT_h = nc.declare_dram_parameter("xT", [D, L], BF16, isOutput=False)
    wqkv_h = nc.declare_dram_parameter("wqkv", [D, 3 * D], BF16, isOutput=False)
    bqkv_h = nc.declare_dram_parameter("bqkv2", [128, 18], F32, isOutput=False)
    wp_h = nc.declare_dram_parameter("wproj2", [128, DT, D], BF16, isOutput=False)
    bp_h = nc.declare_dram_parameter("bproj", [D], F32, isOutput=False)
    bv_h = nc.declare_dram_parameter("bvvec", [D], F32, isOutput=False)
    y_h = nc.declare_dram_parameter("y", [LQ, D], F32, isOutput=True)

    with tile.TileContext(nc) as tc:
        with tc.tile_pool(name="persist", bufs=1) as pp:
            KT_sb = pp.tile([128, DT, L], BF16)    # K^T per pair c-tile
            QT2_sb = pp.tile([128, DT, LQ], BF16)  # Q^T packed pairs
            V_sb = pp.tile([128, LT, VPAD], BF16)  # [V_h | ones] blocks at h*65
            OT2_sb = pp.tile([128, DT, LQ], BF16)  # ctx^T per pair (packed)
            bias_sb = pp.tile([128, 18], F32)
            bv_sb = pp.tile([128, D], F32)
            bp_sb = pp.tile([128, D], F32)
            dst_sb = pp.tile([128, LQ], F32)       # denom staging rows 63/64
            DallA = pp.tile([1, LQ], F32)          # pair denominators (rotating)
            DallB = pp.tile([1, LQ], F32)
            RsbA = pp.tile([1, LQ], F32)           # 1/denom
            RsbB = pp.tile([1, LQ], F32)
            rbA_sb = pp.tile([128, LQ], F32)       # bcast 1/denom (full rows)
            rbB_sb = pp.tile([128, LQ], F32)
            wp_sb = pp.tile([128, DT, D], BF16)

            # partition_broadcast needs the attn gpsimd ucode library
            nc.gpsimd.load_library(library_config.attn)
            # constants
            for h in range(H):
                nc.vector.memset(V_sb[:, :, h * VW + HD:h * VW + HD + 1], 1.0)
            nc.vector.memset(dst_sb[0:1, 0:8], 0.0)
            # preload the exp table set during input DMA
            nc.scalar.activation(dst_sb[0:1, 4:8], dst_sb[0:1, 0:4], Act.Exp)

            with (
                tc.tile_pool(name="ptp", bufs=2) as ptp,
                tc.tile_pool(name="ps_s", bufs=2, space="PSUM") as ps_s,
                tc.tile_pool(name="ps_av", bufs=2, space="PSUM") as ps_av,
            ):
                wq_r = wqkv_h[:].rearrange("(n p) c -> p n c", p=128)
                xT_r = xT_h[:].rearrange("(n p) l -> p n l", p=128)

                def qT_block(kt):
                    ps = ps_s.tile([128, 2, LQ], F32, tag="sps")
                    for dt in range(DT):
                        nc.tensor.matmul(
                            ps[:, 0, :],
                            wqkv_sb[:, dt, kt * 128:(kt + 1) * 128],
                            xT_sb[:, dt, 0:LQ],
                            start=(dt == 0), stop=(dt == DT - 1),
                        )
                    nc.vector.tensor_scalar_add(
                        QT2_sb[:, kt, :], ps[:, 0, :], bias_sb[:, kt:kt + 1])

                def kT_block(kt, lc):
                    ps = ps_s.tile([128, 2, LQ], F32, tag="sps")
                    for dt in range(DT):
                        nc.tensor.matmul(
                            ps[:, 0, :],
                            wqkv_sb[:, dt, D + kt * 128:D + (kt + 1) * 128],
                            xT_sb[:, dt, lc * 512:(lc + 1) * 512],
                            start=(dt == 0), stop=(dt == DT - 1),
                        )
                    nc.vector.tensor_scalar_add(
                        KT_sb[:, kt, lc * 512:(lc + 1) * 512], ps[:, 0, :],
                        bias_sb[:, 6 + kt:7 + kt])

                def vh_block(lt, hf):
                    ps = ps_s.tile([128, 2, LQ], F32, tag="sps")
                    for dt in range(DT):
                        nc.tensor.matmul(
                            ps[:, 0, 0:384],
                            xT_sb[:, dt, lt * 128:(lt + 1) * 128],
                            wqkv_sb[:, dt, 2 * D + hf * 384:2 * D + (hf + 1) * 384],
                            start=(dt == 0), stop=(dt == DT - 1),
                        )
                    nc.vector.tensor_tensor(
                        V_sb[:, lt, 390 * hf:390 * hf + 390].rearrange(
                            "p (h c) -> p h c", c=VW)[:, :, 0:HD],
                        ps[:, 0, 0:384].rearrange("p (h d) -> p h d", h=6),
                        bv_sb[:, hf * 384:(hf + 1) * 384].rearrange(
                            "p (h d) -> p h d", h=6),
                        Alu.add,
                    )

                def s_slot(p, jt, PT):
                    # S^T[j, i] for both heads of the pair, concurrently:
                    # head A in array rows 0:64, head B in rows 64:128
                    ps = ps_s.tile([128, 2, LQ], F32, tag="sps")
                    nc.tensor.matmul(
                        ps[:, 0, :],
                        KT_sb[0:64, p, jt * 128:(jt + 1) * 128],
                        QT2_sb[0:64, p, :],
                        start=True, stop=True, tile_position=(0, 0),
                    )
                    nc.tensor.matmul(
                        ps[:, 1, :],
                        KT_sb[64:128, p, jt * 128:(jt + 1) * 128],
                        QT2_sb[64:128, p, :],
                        start=True, stop=True, tile_position=(64, 0),
                    )
                    nc.scalar.activation(PT[:, jt, :, :], ps, Act.Exp, scale=SCALE)

                def av_mms(p, jt, PT, av):
                    # ones-column windows: even head ctx rows 0:64 + den row 64;
                    # odd head (window shifted 64 left) ctx rows 64:128 + den row 63
                    nc.tensor.matmul(
                        av[:, 0, :],
                        V_sb[:, jt, 2 * p * VW:2 * p * VW + 128],
                        PT[:, jt, 0, :],
                        start=(jt == 0), stop=(jt == LT - 1),
                        skip_group_check=True,
                    )
                    nc.tensor.matmul(
                        av[:, 1, :],
                        V_sb[:, jt, (2 * p + 1) * VW - 64:(2 * p + 1) * VW + 64],
                        PT[:, jt, 1, :],
                        start=(jt == 0), stop=(jt == LT - 1),
                        skip_group_check=True,
                    )

                def av_finish(p, av):
                    nc.vector.tensor_copy(OT2_sb[0:64, p, :], av[0:64, 0, :])
                    nc.vector.tensor_copy(OT2_sb[64:128, p, :], av[64:128, 1, :])
                    nc.vector.tensor_copy(dst_sb[64:65, :], av[64:65, 0, :])
                    # DVE partition base must be 32-aligned; den row 63 rides
                    # along in a 32:64 copy, the DMA below picks out row 63
                    nc.vector.tensor_copy(dst_sb[32:64, :], av[32:64, 1, :])
                    nc.sync.dma_start(out=DallA[:], in_=dst_sb[64:65, :])
                    nc.sync.dma_start(out=DallB[:], in_=dst_sb[63:64, :])

                def normalize(p):
                    # partition_broadcast must target a full 128-row buffer
                    # (partial-range outputs misbehave on HW ucode)
                    nc.vector.reciprocal_approx_fast(RsbA[:], DallA[:])
                    nc.vector.reciprocal_approx_fast(RsbB[:], DallB[:])
                    nc.gpsimd.partition_broadcast(rbA_sb, RsbA[:])
                    nc.gpsimd.partition_broadcast(rbB_sb, RsbB[:])
                    nc.vector.tensor_tensor(
                        OT2_sb[0:64, p, :], OT2_sb[0:64, p, :],
                        rbA_sb[0:64, :], Alu.mult)
                    nc.vector.tensor_tensor(
                        OT2_sb[64:128, p, :], OT2_sb[64:128, p, :],
                        rbB_sb[64:128, :], Alu.mult)

                def proj_group(ic, eh, p0, p1, first):
                    ps = ps_s.tile([128, 2, LQ], F32, tag="sps")
                    for p in range(p0, p1):
                        nc.tensor.matmul(
                            ps[:, 0, 0:384],
                            OT2_sb[:, p, ic * 128:(ic + 1) * 128],
                            wp_sb[:, p, eh * 384:(eh + 1) * 384],
                            start=(p == p0), stop=(p == p1 - 1),
                        )
                    dst = yacc_sb[:, ic, eh * 384:(eh + 1) * 384]
                    if first:
                        nc.vector.tensor_tensor(
                            dst, ps[:, 0, 0:384],
                            bp_sb[:, eh * 384:(eh + 1) * 384], Alu.add)
                    else:
                        nc.vector.tensor_tensor(dst, dst, ps[:, 0, 0:384], Alu.add)

                # ---- phase A: load pool + qkv GEMMs + pairs 0-2 ----
                with tc.tile_pool(name="loadp", bufs=1) as lp:
                    xT_sb = lp.tile([128, DT, L], BF16)
                    wqkv_sb = lp.tile([128, DT, 3 * D], BF16)

                    # DMA priority order (sync queue is FIFO)
                    nc.sync.dma_start(out=bias_sb, in_=bqkv_h[:])
                    nc.sync.dma_start(
                        out=wqkv_sb[:, :, 0:128], in_=wq_r[:, :, 0:128])
                    nc.sync.dma_start(
                        out=xT_sb[:, :, 0:512], in_=xT_r[:, :, 0:512])
                    nc.sync.dma_start(
                        out=wqkv_sb[:, :, D:D + 128], in_=wq_r[:, :, D:D + 128])
                    nc.sync.dma_start(
                        out=xT_sb[:, :, 512:1024], in_=xT_r[:, :, 512:1024])
                    nc.sync.dma_start(
                        out=wqkv_sb[:, :, 128:D], in_=wq_r[:, :, 128:D])
                    nc.sync.dma_start(
                        out=wqkv_sb[:, :, D + 128:2 * D],
                        in_=wq_r[:, :, D + 128:2 * D])
                    nc.sync.dma_start(
                        out=wqkv_sb[:, :, 2 * D:3 * D], in_=wq_r[:, :, 2 * D:3 * D])
                    nc.sync.dma_start(
                        out=xT_sb[:, :, 1024:1536], in_=xT_r[:, :, 1024:1536])
                    nc.sync.dma_start(
                        out=xT_sb[:, :, 1536:2048], in_=xT_r[:, :, 1536:2048])
                    nc.sync.dma_start(out=wp_sb, in_=wp_h[:])
                    bv_src = bv_h[:]
                    nc.gpsimd.dma_start(
                        out=bv_sb,
                        in_=bass.AP(tensor=bv_src.tensor, offset=bv_src.offset,
                                    ap=[[0, 128]] + list(bv_src.ap)),
                    )
                    bp_src = bp_h[:]
                    nc.gpsimd.dma_start(
                        out=bp_sb,
                        in_=bass.AP(tensor=bp_src.tensor, offset=bp_src.offset,
                                    ap=[[0, 128]] + list(bp_src.ap)),
                    )

                    qT_block(0)
                    kT_block(0, 0)

                    # filler thunks per slot for pairs 0-2 (DMA-arrival order)
                    def F(*thunks):
                        return list(thunks)

                    fillers = {}
                    fillers[0] = [
                        F(lambda lt=lt: vh_block(lt, 0), lambda lt=lt: vh_block(lt, 1))
                        for lt in range(4)
                    ]
                    fillers[0][3] += [lambda: kT_block(0, 1)]
                    fillers[0] += [
                        F(lambda: vh_block(4, 0), lambda: vh_block(4, 1)),
                        F(lambda: vh_block(5, 0), lambda: vh_block(5, 1)),
                        F(lambda: vh_block(6, 0), lambda: vh_block(6, 1), lambda: qT_block(1)),
                        F(lambda: vh_block(7, 0), lambda: vh_block(7, 1),
                          lambda: kT_block(0, 2)),
                        F(lambda: qT_block(2), lambda: qT_block(3)),
                        F(lambda: vh_block(8, 0), lambda: vh_block(8, 1), lambda: qT_block(4)),
                        F(lambda: vh_block(9, 0), lambda: vh_block(9, 1), lambda: qT_block(5)),
                        F(lambda: vh_block(10, 0), lambda: vh_block(10, 1), lambda: kT_block(0, 3)),
                        F(lambda: vh_block(11, 0), lambda: vh_block(11, 1), lambda: kT_block(1, 0)),
                        F(lambda: vh_block(12, 0), lambda: vh_block(12, 1), lambda: kT_block(1, 1)),
                        F(lambda: vh_block(13, 0), lambda: vh_block(13, 1), lambda: kT_block(1, 2)),
                        F(lambda: vh_block(14, 0), lambda: vh_block(14, 1), lambda: kT_block(1, 3)),
                    ]
                    fillers[1] = [F() for _ in range(LT)]
                    fillers[1][0] = F(lambda: vh_block(15, 0), lambda: vh_block(15, 1))
                    for i, (kt, lc) in enumerate(
                            [(2, lc) for lc in range(4)] + [(3, lc) for lc in range(4)]):
                        fillers[1][2 * i + 1] += [lambda kt=kt, lc=lc: kT_block(kt, lc)]
                    fillers[2] = [F() for _ in range(LT)]
                    for i, (kt, lc) in enumerate(
                            [(4, lc) for lc in range(4)] + [(5, lc) for lc in range(4)]):
                        fillers[2][2 * i] += [lambda kt=kt, lc=lc: kT_block(kt, lc)]

                    PTs = {}
                    avs = {}
                    for p in range(3):
                        PTs[p] = ptp.tile([128, LT, 2, LQ], BF16, tag="PT", name=f"PT{p}")
                        if p >= 1:
                            avs[p - 1] = ps_av.tile([128, 2, LQ], F32, tag="av", name=f"av{p-1}")
                        for jt in range(LT):
                            s_slot(p, jt, PTs[p])
                            if p >= 1:
                                av_mms(p - 1, jt, PTs[p - 1], avs[p - 1])
                            for thunk in fillers[p][jt]:
                                thunk()
                        if p >= 1:
                            av_finish(p - 1, avs[p - 1])
                            normalize(p - 1)

                # ---- phase B: pairs 3-5 + projection (xT/wqkv space freed) ----
                with tc.tile_pool(name="yaccp", bufs=1) as yp:
                    yacc_sb = yp.tile([128, 4, D], F32)

                    for p in range(3, NP):
                        PTs[p] = ptp.tile([128, LT, 2, LQ], BF16, tag="PT", name=f"PT{p}")
                        avs[p - 1] = ps_av.tile([128, 2, LQ], F32, tag="av", name=f"av{p-1}")
                        if p == NP - 1:
                            avs[p] = ps_av.tile([128, 2, LQ], F32, tag="av", name=f"av{p}")
                        pg = 0
                        for jt in range(LT):
                            s_slot(p, jt, PTs[p])
                            av_mms(p - 1, jt, PTs[p - 1], avs[p - 1])
                            if p == NP - 1 and jt >= 1:
                                av_mms(p, jt - 1, PTs[p], avs[p])
                            if p == NP - 1 and jt % 2 == 0:
                                # pairs 0-3 projection woven into pair 5
                                proj_group(pg // 2, pg % 2, 0, 4, first=True)
                                pg += 1
                        av_finish(p - 1, avs[p - 1])
                        normalize(p - 1)

                    # tail: finish av(5), normalize, project pairs 4-5, DMA out
                    p = NP - 1
                    av_mms(p, LT - 1, PTs[p], avs[p])
                    av_finish(p, avs[p])
                    normalize(p)
                    y_r = y_h[:].rearrange("(n p) e -> p n e", p=128)
                    for ic in range(4):
                        for eh in range(2):
                            proj_group(ic, eh, 4, 6, first=False)
                        nc.sync.dma_start(out=y_r[:, ic, :], in_=yacc_sb[:, ic, :])

    nc.finalize()
    return nc


def _get_nc():
    if "nc" not in _CACHED:
        _CACHED["nc"] = _build_nc()
    return _CACHED["nc"]


def _make_in_maps(x, Wqkv, bqkv, Wproj, bproj):
    bf16 = ml_dtypes.bfloat16
    x = np.asarray(x, dtype=np.float32)
    wqkv16 = np.ascontiguousarray(np.asarray(Wqkv, dtype=np.float32).astype(bf16))
    bqkv32 = np.asarray(bqkv, dtype=np.float32)
    bqkv2 = np.ascontiguousarray(bqkv32.reshape(18, 128).T)
    bvvec = np.ascontiguousarray(bqkv32[2 * D:3 * D])
    wp2 = np.ascontiguousarray(
        np.asarray(Wproj, dtype=np.float32).astype(bf16)
        .reshape(D // 128, 128, D).transpose(1, 0, 2))
    bp32 = np.ascontiguousarray(np.asarray(bproj, dtype=np.float32))

    xT = [np.ascontiguousarray(x[b].T.astype(bf16)) for b in range(B)]
    in_maps = []
    for c in range(N_CORES):
        b, q = c // 4, c % 4
        # rotate so this core's query quarter is first; softmax over keys is
        # order-invariant, so the rotated j-order is used consistently
        xrot = np.ascontiguousarray(
            np.concatenate([xT[b][:, q * LQ:], xT[b][:, :q * LQ]], axis=1))
        in_maps.append({
            "xT": xrot,
            "wqkv": wqkv16,
            "bqkv2": bqkv2,
            "bvvec": bvvec,
            "wproj2": wp2,
            "bproj": bp32,
        })
    return in_maps


def run(inputs, trace=False):
    """Run the SPMD kernel. Returns (full output [2,2048,768] f32, results)."""
    from concourse.bass_utils import run_bass_kernel_spmd

    nc = _get_nc()
    in_maps = _make_in_maps(**inputs)
    res = run_bass_kernel_spmd(nc, in_maps, list(range(N_CORES)), trace=trace)
    out = np.empty((B, L, D), dtype=np.float32)
    for c in range(N_CORES):
        b, q = c // 4, c % 4
        out[b, q * LQ:(q + 1) * LQ, :] = res.results[c]["y"]
    return out, res


def kernel(**inputs) -> np.ndarray:
    return run(inputs)[0]


# revision 20
# speedup vs baseline: 1.0217x; 1.0135x over previous
"""Distributed multi-head attention kernel for 8 Trainium2 NeuronCores.

Problem: x[2,2048,768] @ Wqkv[768,2304] + bqkv -> 12-head attention -> @ Wproj + bproj.

Sharding: data-parallel over batch (2) x sequence-quarter (4) = 8 cores, no
collectives. Each core receives xT ROTATED so its own query quarter is the
first 512 columns; softmax over keys is order-invariant, so K/V/scores use
the rotated j-order uniformly (no per-core program divergence).

ScalarE exp is the pacing engine (12 heads x 512 x 2048 exps ~ 110us floor at
1 elem/cycle/lane); the schedule keeps it fed from ~5us on:
- Scores run ROW-PACKED: per head pair one PSUM slot gets two concurrent
  64-contract matmuls at tile_position (0,0)/(64,0) (head A in array rows
  0:64, head B in 64:128), so score MMs cost half the baseline.
- Each exp instruction covers both heads of a pair for one j-tile
  ([128, 2, 512] PSUM -> PT bf16).
- The attention-value matmul keeps the baseline's ones-column trick (V window
  of 128 cols gives 64 ctx rows + denominator row per head, junk never read),
  accumulated per pair into one 2-bank PSUM tile, lagged one pair behind exp
  (within-pair lag 2 for the last pair to kill the tail).
- Denominators: staging row copies + DMA gather to [12, 512], then per-pair
  reciprocal_approx_fast + GpSimd partition_broadcast + one [128,512] DVE
  mult on the packed pair context (normalize-late; projection is linear).
- Projection: pairs 0-3 contraction woven into pair 5 (into an SBUF f32
  accumulator that reuses the freed xT/wqkv space), pairs 4-5 in the tail.
- DMA priority order + lazy K^T/Q^T c-tile emission keep the first exp at
  ~5us; V blocks are the main pair-0/1 filler; K^T c-tiles finish by pair 2
  so the xT/wqkv pool can close.
"""

import numpy as np
import ml_dtypes

B = 2
L = 2048
D = 768
H = 12
HD = 64
SCALE = HD ** -0.5
N_CORES = 8
LQ = L // 4   # 512 query rows per core
LT = L // 128  # 16 key tiles
DT = D // 128  # 6 contraction tiles
NP = H // 2    # 6 head pairs
VW = 65        # V block width per head (64 ctx + 1 ones)
VPAD = 11 * VW + 128 + 5

_CACHED = {}


def _build_nc():
    import concourse.bass as bass
    import concourse.mybir as mybir
    import concourse.tile as tile
    from concourse import bacc, library_config

    F32 = mybir.dt.float32
    BF16 = mybir.dt.bfloat16
    Alu = mybir.AluOpType
    Act = mybir.ActivationFunctionType

    nc = bacc.Bacc(target_bir_lowering=False)

    xT_h = nc.declare_dram_parameter("xT", [D, L], BF16, isOutput=False)
    wqkv_h = nc.declare_dram_parameter("wqkv", [D, 3 * D], BF16, isOutput=False)
    bqkv_h = nc.declare_dram_parameter("bqkv", [3 * D], F32, isOutput=False)
    wp_h = nc.declare_dram_parameter("wproj2", [128, DT, D], BF16, isOutput=False)
    bp_h = nc.declare_dram_parameter("bproj", [D], F32, isOutput=False)
    y_h = nc.declare_dram_parameter("y", [LQ, D], F32, isOutput=True)

    with tile.TileContext(nc) as tc:
        with tc.tile_pool(name="persist", bufs=1) as pp:
            KT_sb = pp.tile([128, DT, L], BF16)    # K^T per pair c-tile
            QT2_sb = pp.tile([128, DT, LQ], BF16)  # Q^T packed pairs
            V_sb = pp.tile([128, LT, VPAD], BF16)  # [V_h | ones] blocks at h*65
            OT2_sb = pp.tile([128, DT, LQ], BF16)  # ctx^T per pair (packed)
            bias_sb = pp.tile([128, 18], F32)
            bv_sb = pp.tile([128, D], F32)
            bp_sb = pp.tile([128, D], F32)
            dst_sb = pp.tile([128, LQ], F32)       # denom staging rows 63/64
            DallA = pp.tile([1, LQ], F32)          # pair denominators (rotating)
            DallB = pp.tile([1, LQ], F32)
            RsbA = pp.tile([1, LQ], F32)           # 1/denom
            RsbB = pp.tile([1, LQ], F32)
            rbA_sb = pp.tile([128, LQ], F32)       # bcast 1/denom (full rows)
            rbB_sb = pp.tile([128, LQ], F32)
            wp_sb = pp.tile([128, DT, D], BF16)

            # partition_broadcast needs the attn gpsimd ucode library
            nc.gpsimd.load_library(library_config.attn)
            # constants
            for h in range(H):
                nc.vector.memset(V_sb[:, :, h * VW + HD:h * VW + HD + 1], 1.0)
            nc.vector.memset(dst_sb[0:1, 0:8], 0.0)
            # preload the exp table set during input DMA
            nc.scalar.activation(dst_sb[0:1, 4:8], dst_sb[0:1, 0:4], Act.Exp)

            with (
                tc.tile_pool(name="ptp", bufs=2) as ptp,
                tc.tile_pool(name="ps_s", bufs=2, space="PSUM") as ps_s,
                tc.tile_pool(name="ps_av", bufs=2, space="PSUM") as ps_av,
            ):
                wq_r = wqkv_h[:].rearrange("(n p) c -> p n c", p=128)
                xT_r = xT_h[:].rearrange("(n p) l -> p n l", p=128)

                def qT_block(kt):
                    ps = ps_s.tile([128, 2, LQ], F32, tag="sps")
                    for dt in range(DT):
                        nc.tensor.matmul(
                            ps[:, 0, :],
                            wqkv_sb[:, dt, kt * 128:(kt + 1) * 128],
                            xT_sb[:, dt, 0:LQ],
                            start=(dt == 0), stop=(dt == DT - 1),
                        )
                    nc.vector.tensor_scalar_add(
                        QT2_sb[:, kt, :], ps[:, 0, :], bias_sb[:, kt:kt + 1])

                def kT_block(kt, lc):
                    ps = ps_s.tile([128, 2, LQ], F32, tag="sps")
                    for dt in range(DT):
                        nc.tensor.matmul(
                            ps[:, 0, :],
                            wqkv_sb[:, dt, D + kt * 128:D + (kt + 1) * 128],
                            xT_sb[:, dt, lc * 512:(lc + 1) * 512],
                            start=(dt == 0), stop=(dt == DT - 1),
                        )
                    nc.vector.tensor_scalar_add(
                        KT_sb[:, kt, lc * 512:(lc + 1) * 512], ps[:, 0, :],
                        bias_sb[:, 6 + kt:7 + kt])

                def vh_block(lt, hf):
                    ps = ps_s.tile([128, 2, LQ], F32, tag="sps")
                    for dt in range(DT):
                        nc.tensor.matmul(
                            ps[:, 0, 0:384],
                            xT_sb[:, dt, lt * 128:(lt + 1) * 128],
                            wqkv_sb[:, dt, 2 * D + hf * 384:2 * D + (hf + 1) * 384],
                            start=(dt == 0), stop=(dt == DT - 1),
                        )
                    nc.vector.tensor_tensor(
                        V_sb[:, lt, 390 * hf:390 * hf + 390].rearrange(
                            "p (h c) -> p h c", c=VW)[:, :, 0:HD],
                        ps[:, 0, 0:384].rearrange("p (h d) -> p h d", h=6),
                        bv_sb[:, hf * 384:(hf + 1) * 384].rearrange(
                            "p (h d) -> p h d", h=6),
                        Alu.add,
                    )

                def s_slot(p, jt, PT):
                    # S^T[j, i] for both heads of the pair, concurrently:
                    # head A in array rows 0:64, head B in rows 64:128
                    ps = ps_s.tile([128, 2, LQ], F32, tag="sps")
                    nc.tensor.matmul(
                        ps[:, 0, :],
                        KT_sb[0:64, p, jt * 128:(jt + 1) * 128],
                        QT2_sb[0:64, p, :],
                        start=True, stop=True, tile_position=(0, 0),
                    )
                    nc.tensor.matmul(
                        ps[:, 1, :],
                        KT_sb[64:128, p, jt * 128:(jt + 1) * 128],
                        QT2_sb[64:128, p, :],
                        start=True, stop=True, tile_position=(64, 0),
                    )
                    nc.scalar.activation(PT[:, jt, :, :], ps, Act.Exp, scale=SCALE)

                def av_mms(p, jt, PT, av):
                    # ones-column windows: even head ctx rows 0:64 + den row 64;
                    # odd head (window shifted 64 left) ctx rows 64:128 + den row 63
                    nc.tensor.matmul(
                        av[:, 0, :],
                        V_sb[:, jt, 2 * p * VW:2 * p * VW + 128],
                        PT[:, jt, 0, :],
                        start=(jt == 0), stop=(jt == LT - 1),
                        skip_group_check=True,
                    )
                    nc.tensor.matmul(
                        av[:, 1, :],
                        V_sb[:, jt, (2 * p + 1) * VW - 64:(2 * p + 1) * VW + 64],
                        PT[:, jt, 1, :],
                        start=(jt == 0), stop=(jt == LT - 1),
                        skip_group_check=True,
                    )

                def av_finish(p, av):
                    nc.vector.tensor_copy(OT2_sb[0:64, p, :], av[0:64, 0, :])
                    nc.vector.tensor_copy(OT2_sb[64:128, p, :], av[64:128, 1, :])
                    nc.vector.tensor_copy(dst_sb[64:65, :], av[64:65, 0, :])
                    # DVE partition base must be 32-aligned; den row 63 rides
                    # along in a 32:64 copy, the DMA below picks out row 63
                    nc.vector.tensor_copy(dst_sb[32:64, :], av[32:64, 1, :])
                    nc.sync.dma_start(out=DallA[:], in_=dst_sb[64:65, :])
                    nc.sync.dma_start(out=DallB[:], in_=dst_sb[63:64, :])

                def normalize(p):
                    # partition_broadcast must target a full 128-row buffer
                    # (partial-range outputs misbehave on HW ucode)
                    nc.vector.reciprocal_approx_fast(RsbA[:], DallA[:])
                    nc.vector.reciprocal_approx_fast(RsbB[:], DallB[:])
                    nc.gpsimd.partition_broadcast(rbA_sb, RsbA[:])
                    nc.gpsimd.partition_broadcast(rbB_sb, RsbB[:])
                    nc.vector.tensor_tensor(
                        OT2_sb[0:64, p, :], OT2_sb[0:64, p, :],
                        rbA_sb[0:64, :], Alu.mult)
                    nc.vector.tensor_tensor(
                        OT2_sb[64:128, p, :], OT2_sb[64:128, p, :],
                        rbB_sb[64:128, :], Alu.mult)

                def proj_group(ic, eh, p0, p1, first):
                    ps = ps_s.tile([128, 2, LQ], F32, tag="sps")
                    for p in range(p0, p1):
                        nc.tensor.matmul(
                            ps[:, 0, 0:384],
                            OT2_sb[:, p, ic * 128:(ic + 1) * 128],
                            wp_sb[:, p, eh * 384:(eh + 1) * 384],
                            start=(p == p0), stop=(p == p1 - 1),
                        )
                    dst = yacc_sb[:, ic, eh * 384:(eh + 1) * 384]
                    if first:
                        nc.vector.tensor_tensor(
                            dst, ps[:, 0, 0:384],
                            bp_sb[:, eh * 384:(eh + 1) * 384], Alu.add)
                    else:
                        nc.vector.tensor_tensor(dst, dst, ps[:, 0, 0:384], Alu.add)

                # ---- phase A: load pool + qkv GEMMs + pairs 0-2 ----
                with tc.tile_pool(name="loadp", bufs=1) as lp:
                    xT_sb = lp.tile([128, DT, L], BF16)
                    wqkv_sb = lp.tile([128, DT, 3 * D], BF16)

                    # DMA priority order (sync queue is FIFO)
                    nc.sync.dma_start(
                        out=bias_sb, in_=bqkv_h[:].rearrange("(n p) -> p n", p=128))
                    nc.sync.dma_start(
                        out=wqkv_sb[:, :, 0:128], in_=wq_r[:, :, 0:128])
                    nc.sync.dma_start(
                        out=xT_sb[:, :, 0:512], in_=xT_r[:, :, 0:512])
                    nc.sync.dma_start(
                        out=wqkv_sb[:, :, D:D + 128], in_=wq_r[:, :, D:D + 128])
                    nc.sync.dma_start(
                        out=wqkv_sb[:, :, 2 * D:3 * D], in_=wq_r[:, :, 2 * D:3 * D])
                    nc.sync.dma_start(
                        out=xT_sb[:, :, 512:1024], in_=xT_r[:, :, 512:1024])
                    nc.sync.dma_start(
                        out=wqkv_sb[:, :, 128:D], in_=wq_r[:, :, 128:D])
                    nc.sync.dma_start(
                        out=wqkv_sb[:, :, D + 128:2 * D],
                        in_=wq_r[:, :, D + 128:2 * D])
                    nc.sync.dma_start(
                        out=xT_sb[:, :, 1024:1536], in_=xT_r[:, :, 1024:1536])
                    nc.sync.dma_start(
                        out=xT_sb[:, :, 1536:2048], in_=xT_r[:, :, 1536:2048])
                    nc.sync.dma_start(out=wp_sb, in_=wp_h[:])
                    bv_src = bqkv_h[2 * D:3 * D]
                    nc.gpsimd.dma_start(
                        out=bv_sb,
                        in_=bass.AP(tensor=bv_src.tensor, offset=bv_src.offset,
                                    ap=[[0, 128]] + list(bv_src.ap)),
                    )
                    bp_src = bp_h[:]
                    nc.gpsimd.dma_start(
                        out=bp_sb,
                        in_=bass.AP(tensor=bp_src.tensor, offset=bp_src.offset,
                                    ap=[[0, 128]] + list(bp_src.ap)),
                    )

                    qT_block(0)
                    kT_block(0, 0)

                    # filler thunks per slot for pairs 0-2 (DMA-arrival order)
                    def F(*thunks):
                        return list(thunks)

                    fillers = {}
                    fillers[0] = [
                        F(lambda lt=lt: vh_block(lt, 0), lambda lt=lt: vh_block(lt, 1))
                        for lt in range(4)
                    ]
                    fillers[0][3] += [lambda: kT_block(0, 1)]
                    fillers[0] += [
                        F(lambda: vh_block(4, 0), lambda: vh_block(4, 1)),
                        F(lambda: vh_block(5, 0), lambda: vh_block(5, 1)),
                        F(lambda: vh_block(6, 0), lambda: vh_block(6, 1), lambda: qT_block(1)),
                        F(lambda: vh_block(7, 0), lambda: vh_block(7, 1),
                          lambda: kT_block(0, 2)),
                        F(lambda: qT_block(2), lambda: qT_block(3)),
                        F(lambda: vh_block(8, 0), lambda: vh_block(8, 1), lambda: qT_block(4)),
                        F(lambda: vh_block(9, 0), lambda: vh_block(9, 1), lambda: qT_block(5)),
                        F(lambda: vh_block(10, 0), lambda: vh_block(10, 1), lambda: kT_block(0, 3)),
                        F(lambda: vh_block(11, 0), lambda: vh_block(11, 1), lambda: kT_block(1, 0)),
                        F(lambda: vh_block(12, 0), lambda: vh_block(12, 1), lambda: kT_block(1, 1)),
                        F(lambda: vh_block(13, 0), lambda: vh_block(13, 1), lambda: kT_block(1, 2)),
                        F(lambda: vh_block(14, 0), lambda: vh_block(14, 1), lambda: kT_block(1, 3)),
                    ]
                    fillers[1] = [F() for _ in range(LT)]
                    fillers[1][0] = F(lambda: vh_block(15, 0), lambda: vh_block(15, 1))
                    for i, (kt, lc) in enumerate(
                            [(2, lc) for lc in range(4)] + [(3, lc) for lc in range(4)]):
                        fillers[1][2 * i + 1] += [lambda kt=kt, lc=lc: kT_block(kt, lc)]
                    fillers[2] = [F() for _ in range(LT)]
                    for i, (kt, lc) in enumerate(
                            [(4, lc) for lc in range(4)] + [(5, lc) for lc in range(4)]):
                        fillers[2][2 * i] += [lambda kt=kt, lc=lc: kT_block(kt, lc)]

                    PTs = {}
                    avs = {}
                    for p in range(3):
                        PTs[p] = ptp.tile([128, LT, 2, LQ], BF16, tag="PT", name=f"PT{p}")
                        if p >= 1:
                            avs[p - 1] = ps_av.tile([128, 2, LQ], F32, tag="av", name=f"av{p-1}")
                        for jt in range(LT):
                            s_slot(p, jt, PTs[p])
                            if p >= 1:
                                av_mms(p - 1, jt, PTs[p - 1], avs[p - 1])
                            for thunk in fillers[p][jt]:
                                thunk()
                        if p >= 1:
                            av_finish(p - 1, avs[p - 1])
                            normalize(p - 1)

                # ---- phase B: pairs 3-5 + projection (xT/wqkv space freed) ----
                with tc.tile_pool(name="yaccp", bufs=1) as yp:
                    yacc_sb = yp.tile([128, 4, D], F32)

                    for p in range(3, NP):
                        PTs[p] = ptp.tile([128, LT, 2, LQ], BF16, tag="PT", name=f"PT{p}")
                        avs[p - 1] = ps_av.tile([128, 2, LQ], F32, tag="av", name=f"av{p-1}")
                        if p == NP - 1:
                            avs[p] = ps_av.tile([128, 2, LQ], F32, tag="av", name=f"av{p}")
                        pg = 0
                        for jt in range(LT):
                            s_slot(p, jt, PTs[p])
                            av_mms(p - 1, jt, PTs[p - 1], avs[p - 1])
                            if p == NP - 1 and jt >= 2:
                                av_mms(p, jt - 2, PTs[p], avs[p])
                            if p == NP - 1 and jt % 2 == 0:
                                # pairs 0-3 projection woven into pair 5
                                proj_group(pg // 2, pg % 2, 0, 4, first=True)
                                pg += 1
                        av_finish(p - 1, avs[p - 1])
                        normalize(p - 1)

                    # tail: finish av(5), normalize, project pairs 4-5, DMA out
                    p = NP - 1
                    av_mms(p, LT - 2, PTs[p], avs[p])
                    av_mms(p, LT - 1, PTs[p], avs[p])
                    av_finish(p, avs[p])
                    normalize(p)
                    y_r = y_h[:].rearrange("(n p) e -> p n e", p=128)
                    for ic in range(4):
                        for eh in range(2):
                            proj_group(ic, eh, 4, 6, first=False)
                        nc.sync.dma_start(out=y_r[:, ic, :], in_=yacc_sb[:, ic, :])

    nc.finalize()
    return nc


def _get_nc():
    if "nc" not in _CACHED:
        _CACHED["nc"] = _build_nc()
    return _CACHED["nc"]


def _make_in_maps(x, Wqkv, bqkv, Wproj, bproj):
    bf16 = ml_dtypes.bfloat16
    x = np.asarray(x, dtype=np.float32)
    wqkv16 = np.ascontiguousarray(np.asarray(Wqkv, dtype=np.float32).astype(bf16))
    bqkv32 = np.ascontiguousarray(np.asarray(bqkv, dtype=np.float32))
    wp2 = np.ascontiguousarray(
        np.asarray(Wproj, dtype=np.float32).astype(bf16)
        .reshape(D // 128, 128, D).transpose(1, 0, 2))
    bp32 = np.ascontiguousarray(np.asarray(bproj, dtype=np.float32))

    xT = [np.ascontiguousarray(x[b].T.astype(bf16)) for b in range(B)]
    in_maps = []
    for c in range(N_CORES):
        b, q = c // 4, c % 4
        # rotate so this core's query quarter is first; softmax over keys is
        # order-invariant, so the rotated j-order is used consistently
        xrot = np.ascontiguousarray(
            np.concatenate([xT[b][:, q * LQ:], xT[b][:, :q * LQ]], axis=1))
        in_maps.append({
            "xT": xrot,
            "wqkv": wqkv16,
            "bqkv": bqkv32,
            "wproj2": wp2,
            "bproj": bp32,
        })
    return in_maps


def run(inputs, trace=False):
    """Run the SPMD kernel. Returns (full output [2,2048,768] f32, results)."""
    from concourse.bass_utils import run_bass_kernel_spmd

    nc = _get_nc()
    in_maps = _make_in_maps(**inputs)
    res = run_bass_kernel_spmd(nc, in_maps, list(range(N_CORES)), trace=trace)
    out = np.empty((B, L, D), dtype=np.float32)
    for c in range(N_CORES):
        b, q = c // 4, c % 4
        out[b, q * LQ:(q + 1) * LQ, :] = res.results[c]["y"]
    return out, res


def kernel(**inputs) -> np.ndarray:
    return run(inputs)[0]
